# revision 23
# baseline (speedup 1.0000x reference)
"""Trainium2 Bass kernel for DeformableSincConv1d.

Strategy (data parallel over batch, 4 rows per core on 8 cores):
  1. Windowed im2col load: XX[l, j] = x_pad[10*l + j], j in [0,53)  (l-major chunks of 128)
  2. PE transposes -> XXk[j, l] (k-major), j=0..52
  3. Aligned copies X0 = XXk[1:52], Xp = XXk[2:53] (SBUF->SBUF DMA); Xm = XXk[0:51] view
  4. Offset conv as matmul (contraction over k_in=51), fp32-exact q = fl(TK + off) - TK
  5. Lerp sampling: D = X0 + q*E, E = (Xp-X0) if q>=0 else (X0-Xm)  [deformed, k-major]
  6. Final conv via 51 rotated-filter matmuls (t0-residue decomposition, K=102 stacked
     contraction using a column-shifted copy of D at partitions 51..101)
  7. PSUM evacuation transposes t0-interleaving so the DRAM write is contiguous
"""

import os
import sys

import numpy as np

if "/opt/trn_rl_repo" not in sys.path:
    sys.path.insert(0, "/opt/trn_rl_repo")

SR = 16000
C_OUT = 80
K = 51
STRIDE = 10
HALF = (K - 1) // 2

B_FULL = 32
N_CORES = 8
B_LOC = B_FULL // N_CORES
L_FULL = 32000


def _derive(L):
    L_out = (L - K) // STRIDE + 1
    T_out = (L_out * K - K) // STRIDE + 1
    NCHUNK = (L_out + 127) // 128
    LPAD = NCHUNK * 128
    XLEN = 10 * LPAD + 48  # left pad 1 + right pad; multiple of 16
    return L_out, T_out, NCHUNK, LPAD, XLEN


def _host_filters(hz, band):
    hzc = np.clip(hz.astype(np.float32), 0.0, SR / 2).astype(np.float32)
    bandc = np.clip(band.astype(np.float32), 3.0, SR / 2).astype(np.float32)
    t_right = (np.arange(1, HALF + 1, dtype=np.float32) / np.float32(SR)).astype(np.float32)
    low = (hzc - bandc / 2).astype(np.float32)
    high = (hzc + bandc / 2).astype(np.float32)

    def sinc(t):
        ts = np.where(t == 0, np.float32(1.0), t)
        return np.where(t == 0, np.float32(1.0), np.sin(ts) / ts).astype(np.float32)

    a1 = (2 * high).astype(np.float32)
    a2 = (2 * low).astype(np.float32)
    bp_left = (a1 * sinc(a1 * t_right) - a2 * sinc(a2 * t_right)).astype(np.float32)
    bp = np.concatenate([bp_left, np.ones((C_OUT, 1), np.float32), bp_left[:, ::-1]], axis=1)
    return (bp / (2 * bandc)).astype(np.float32)  # [C_OUT, K]


def _host_f102(filt, L):
    """Stacked rotated filter matrices: [128, K*C_OUT]; rows 0..50 = A-half (column
    offset a), rows 51..101 = B-half (column offset a+1, served by the shifted D copy)."""
    L_out, T_out, _, _, _ = _derive(L)
    F = np.zeros((128, K, C_OUT), np.float32)
    for t0 in range(K):
        a = (STRIDE * t0) // K
        ns = (T_out - t0 + K - 1) // K
        for k2 in range(K):
            kstar = (k2 + STRIDE * t0) % K
            lstar = (STRIDE * t0 + k2) // K
            if lstar == a:
                F[kstar, t0, :] = filt[:, k2]
            else:
                assert lstar == a + 1
                F[51 + kstar, t0, :] = filt[:, k2]
        # bounds: B-half reads D2 col a + 10*(ns-1) which is D col a+1+10*(ns-1)
        assert a + 1 + STRIDE * (ns - 1) <= L_out - 1
    return F.reshape(128, K * C_OUT)


def build_program(B_loc=B_LOC, L=L_FULL, debug=False):
    import concourse.bacc as bacc
    import concourse.tile as tile
    from concourse import bass, mybir

    f32 = mybir.dt.float32
    f32r = mybir.dt.float32r
    i32 = mybir.dt.int32
    Alu = mybir.AluOpType
    Act = mybir.ActivationFunctionType

    L_out, T_out, NCHUNK, LPAD, XLEN = _derive(L)
    CC = 512
    NCC = (L_out + CC - 1) // CC
    NSMAX = (T_out + K - 1) // K

    nc = bacc.Bacc("TRN2", target_bir_lowering=False, debug=debug)

    x_d = nc.dram_tensor("x", [B_loc, XLEN], f32, kind="ExternalInput")
    wr_d = nc.dram_tensor("wr", [K, K], f32, kind="ExternalInput")
    offb_d = nc.dram_tensor("offb", [K, 1], f32, kind="ExternalInput")
    lovec_d = nc.dram_tensor("lovec", [K, 1], f32, kind="ExternalInput")
    hivec_d = nc.dram_tensor("hivec", [K, 1], f32, kind="ExternalInput")
    f102_d = nc.dram_tensor("f102", [128, K * C_OUT], f32r, kind="ExternalInput")
    ident_d = nc.dram_tensor("ident", [128, 128], f32, kind="ExternalInput")
    y_d = nc.dram_tensor("y", [B_loc, C_OUT, T_out], f32, kind="ExternalOutput")

    xap = x_d[:]

    with tile.TileContext(nc) as tc:
        with (
            tc.tile_pool(name="consts", bufs=1) as consts,
            tc.tile_pool(name="xxp", bufs=2) as xxp,
            tc.tile_pool(name="xxkp", bufs=2) as xxkp,
            tc.tile_pool(name="x0p", bufs=1) as x0p,
            tc.tile_pool(name="xpp", bufs=1) as xpp,
            tc.tile_pool(name="ddp", bufs=2) as ddp,
            tc.tile_pool(name="ysbp", bufs=1) as ysbp,
            tc.tile_pool(name="ck", bufs=2) as ck,
            tc.tile_pool(name="tpsum", bufs=3, space="PSUM") as tpsum,
            tc.tile_pool(name="opsum", bufs=2, space="PSUM") as opsum,
            tc.tile_pool(name="fpsum", bufs=3, space="PSUM") as fpsum,
        ):
            wr_sb = consts.tile([K, K], f32)
            nc.sync.dma_start(out=wr_sb[:], in_=wr_d[:])
            offb_sb = consts.tile([K, 1], f32)
            nc.sync.dma_start(out=offb_sb[:], in_=offb_d[:])
            lovec_sb = consts.tile([K, 1], f32)
            nc.sync.dma_start(out=lovec_sb[:], in_=lovec_d[:])
            hivec_sb = consts.tile([K, 1], f32)
            nc.sync.dma_start(out=hivec_sb[:], in_=hivec_d[:])
            f102_sb = consts.tile([128, K * C_OUT], f32r)
            nc.sync.dma_start(out=f102_sb[:], in_=f102_d[:])
            ident_sb = consts.tile([128, 128], f32)
            nc.sync.dma_start(out=ident_sb[:], in_=ident_d[:])

            # TK[k, l] = 10*l + k  (fp32-exact integers)
            tkc = consts.tile([K, L_out], f32)
            for c7 in range(NCC):
                n = min(CC, L_out - c7 * CC)
                sl = slice(c7 * CC, c7 * CC + n)
                it = ck.tile([K, CC], i32, tag="iota")
                nc.gpsimd.iota(it[:, :n], pattern=[[STRIDE, n]],
                               base=STRIDE * CC * c7, channel_multiplier=1)
                nc.vector.tensor_copy(tkc[:, sl], it[:, :n])

            def emit_front(b):
                """Load + transpose + offset conv + sampling for batch row b -> dd tile."""
                xx = xxp.tile([128, NCHUNK, 53], f32)
                nh1 = NCHUNK // 2
                for h0, hn in ((0, nh1), (nh1, NCHUNK - nh1)):
                    in_ap = bass.AP(
                        tensor=xap.tensor,
                        offset=b * XLEN + 1280 * h0,
                        ap=[[10, 128], [1280, hn], [1, 53]],
                    )
                    nc.gpsimd.dma_start(out=xx[:, h0:h0 + hn, :], in_=in_ap)

                xxk = xxkp.tile([53, LPAD], f32)
                for c in range(NCHUNK):
                    pt = tpsum.tile([53, 128], f32)
                    nc.tensor.transpose(pt[:], xx[:, c, :], ident_sb[:])
                    if c % 2 == 0:
                        nc.vector.tensor_copy(xxk[:, c * 128:(c + 1) * 128], pt[:])
                    else:
                        nc.scalar.copy(xxk[:, c * 128:(c + 1) * 128], pt[:])

                x0 = x0p.tile([K, LPAD], f32)
                nc.sync.dma_start(out=x0[:, :L_out], in_=xxk[1:52, :L_out])
                xp = xpp.tile([K, LPAD], f32)
                nc.sync.dma_start(out=xp[:, :L_out], in_=xxk[2:53, :L_out])

                dd = ddp.tile([128, LPAD], f32r)
                nc.gpsimd.memset(dd[0:102, L_out - 1:LPAD].bitcast(f32), 0.0)
                for c7 in range(NCC):
                    n = min(CC, L_out - c7 * CC)
                    sl = slice(c7 * CC, c7 * CC + n)
                    po = opsum.tile([K, CC], f32)
                    nc.tensor.matmul(po[:, :n], wr_sb[:], x0[:, sl],
                                     start=True, stop=True)
                    qt = ck.tile([K, CC], f32, tag="q")
                    tt = ck.tile([K, CC], f32, tag="t")
                    nc.scalar.activation(qt[:, :n], po[:, :n], Act.Identity,
                                         bias=offb_sb[:])
                    nc.gpsimd.tensor_add(tt[:, :n], qt[:, :n], tkc[:, sl])
                    nc.vector.tensor_sub(qt[:, :n], tt[:, :n], tkc[:, sl])
                    nc.vector.tensor_scalar(qt[:, :n], qt[:, :n], lovec_sb[:],
                                            hivec_sb[:], op0=Alu.max, op1=Alu.min)
                    mt = ck.tile([K, CC], mybir.dt.uint8, tag="m")
                    nc.vector.tensor_scalar(mt[:, :n], qt[:, :n], 0.0, None,
                                            op0=Alu.is_ge)
                    et = ck.tile([K, CC], f32, tag="e")
                    dpt = ck.tile([K, CC], f32, tag="dp")
                    nc.vector.tensor_sub(et[:, :n], x0[:, sl], xxk[0:51, sl])
                    nc.vector.tensor_sub(dpt[:, :n], xp[:, sl], x0[:, sl])
                    nc.vector.copy_predicated(et[:, :n], mt[:, :n], dpt[:, :n])
                    nc.vector.tensor_mul(et[:, :n], qt[:, :n], et[:, :n])
                    nc.vector.tensor_add(dd[0:51, sl], x0[:, sl], et[:, :n])

                nc.sync.dma_start(out=dd[51:102, 0:L_out - 1], in_=dd[0:51, 1:L_out])
                return dd

            def emit_final(b, dd):
                ysb = ysbp.tile([C_OUT, T_out], f32)
                for t0 in range(K):
                    a = (STRIDE * t0) // K
                    ns = (T_out - t0 + K - 1) // K
                    nsp = ns + (ns % 2)  # fp32r needs even moving free count
                    fp = fpsum.tile([C_OUT, NSMAX], f32)
                    rhs = dd[0:102, a:a + STRIDE * (nsp - 1) + 1:STRIDE]
                    lhsT = f102_sb[0:102, t0 * C_OUT:(t0 + 1) * C_OUT]
                    nc.tensor.matmul(fp[:, :nsp], lhsT, rhs, start=True, stop=True)
                    nh = ns // 2
                    yv1 = ysb[:, t0:t0 + K * (nh - 1) + 1:K]
                    yv2 = ysb[:, t0 + K * nh:t0 + K * (ns - 1) + 1:K]
                    nc.vector.tensor_copy(yv1, fp[:, :nh])
                    nc.scalar.copy(yv2, fp[:, nh:ns])
                nc.sync.dma_start(out=y_d[b], in_=ysb[:])

            dd_prev = emit_front(0)
            for b in range(B_loc):
                dd_next = emit_front(b + 1) if b + 1 < B_loc else None
                emit_final(b, dd_prev)
                dd_prev = dd_next

    nc.compile()
    return nc


def _host_inputs(x, hz, band, offset_w, offset_b, B_loc, L):
    """Build the per-core input maps."""
    L_out, T_out, NCHUNK, LPAD, XLEN = _derive(L)
    filt = _host_filters(hz, band)
    f102 = _host_f102(filt, L)
    wr = np.ascontiguousarray(offset_w[:, 0, :].T.astype(np.float32))  # [k_in, k_out]
    offb = offset_b.astype(np.float32).reshape(K, 1)
    ident = np.eye(128, dtype=np.float32)

    B = x.shape[0]
    xpad = np.zeros((B, XLEN), np.float32)
    xpad[:, 1:1 + L] = x

    n_cores = B // B_loc
    in_maps = []
    for i in range(n_cores):
        in_maps.append({
            "x": np.ascontiguousarray(xpad[i * B_loc:(i + 1) * B_loc]),
            "wr": wr,
            "offb": offb,
            "lovec": -np.arange(K, dtype=np.float32).reshape(K, 1),
            "hivec": (50.0 - np.arange(K, dtype=np.float32)).reshape(K, 1),
            "f102": f102,
            "ident": ident,
        })
    return in_maps


_CACHED = {}


def _get_program():
    key = (B_LOC, L_FULL)
    if key not in _CACHED:
        _CACHED[key] = build_program(B_LOC, L_FULL)
    return _CACHED[key]


def kernel(x, hz, band, offset_w, offset_b):
    from concourse.bass_utils import run_bass_kernel_spmd

    x = np.asarray(x, dtype=np.float32)
    hz = np.asarray(hz, dtype=np.float32)
    band = np.asarray(band, dtype=np.float32)
    offset_w = np.asarray(offset_w, dtype=np.float32)
    offset_b = np.asarray(offset_b, dtype=np.float32)

    nc = _get_program()
    in_maps = _host_inputs(x, hz, band, offset_w, offset_b, B_LOC, L_FULL)
    res = run_bass_kernel_spmd(nc, in_maps, list(range(N_CORES)))
    outs = [res.results[i]["y"] for i in range(N_CORES)]
    return np.concatenate(outs, axis=0)
